# revision 18
# baseline (speedup 1.0000x reference)
"""CapsuleLayer dynamic-routing kernel for 8 Trainium2 NeuronCores.

Sharding: data-parallel over batch (16 batches/core), weight replicated.
  u_hat[b,c,n,s] = sum_i W[c,n,s,i] * x[b,i,c]   (PE, bf16, block-diag x)
  3 routing iterations; b_ij update takes a mean over the full batch via
  AllReduce (split in channel-halves so it overlaps compute).

v2 design vs baseline:
  - everything bf16 (tol 2e-2; measured rel err ~1e-3): halves DMA + 2x DVE
  - u_hat transposed to [c-part, (b,n,s)] via one SBUF->SBUF DMA per chunk
    (512B runs) instead of a DRAM round-trip
  - iteration-0 s_j computed during production from the staging tiles with a
    delta/16 stationary (PE idle time), output directly [b,(n,s)] (no diag)
  - agreement = DVE tensor_tensor mult + pairwise fold tree (bf16 2x mode);
    3 chunk-mults offloaded to GPSIMD
  - diag extract for s_j via a single strided DVE copy from PSUM
  - v broadcast via DRAM bounce + stride-0 broadcast DMA
  - tiny keep-warm matmuls gated on DVE steps hold the PE clock at max
"""

import sys

sys.path.insert(0, "/opt/trn_rl_repo")

import numpy as np

B, IN_UNIT, IN_CHANNEL = 128, 16, 1152
NUM_UNIT, UNIT_SIZE = 16, 16
NCORES = 8
BL = B // NCORES               # 16 batches per core
NCHUNK = IN_CHANNEL // 128     # 9 c-chunks
NGRP = 16                      # groups of 8 channels per chunk
NS = NUM_UNIT * UNIT_SIZE      # 256
FREE = BL * NS                 # 4096 = (b, n, s) free size per chunk
POOL_CHUNKS = (6, 7, 8)        # agreement mults done on gpsimd
JUNK_ON = True                 # keep-warm matmuls
SPLIT_AR = True                # two collectives per round

_cache = {}


def _build(single_core=False, niters=3):
    import concourse.bass as bass
    import concourse.bacc as bacc
    import concourse.mybir as mybir
    import concourse.tile as tile

    f32 = mybir.dt.float32
    bf16 = mybir.dt.bfloat16
    ALU = mybir.AluOpType
    AX = mybir.AxisListType
    ACT = mybir.ActivationFunctionType

    def sub(ap, off, dims, cast=None):
        a = bass.AP(ap.tensor, ap.offset + off, [list(d) for d in dims])
        return a.bitcast(cast) if cast is not None else a

    nc = bacc.Bacc("TRN2", target_bir_lowering=False, debug=False,
                   num_devices=1 if single_core else NCORES)

    wr_t = nc.dram_tensor("wr", [NCHUNK * NGRP * 128, 256], bf16,
                          kind="ExternalInput")
    xbd_t = nc.dram_tensor("xbd", [NCHUNK, 128, NGRP * 128], bf16,
                           kind="ExternalInput")
    sd_t = nc.dram_tensor("sd", [128, BL], bf16, kind="ExternalInput")
    vout_t = nc.dram_tensor("vout", [NUM_UNIT, BL * UNIT_SIZE], f32,
                            kind="ExternalOutput")

    with tile.TileContext(nc) as tc:
        with tc.tile_pool(name="persist", bufs=1) as persist, \
             tc.tile_pool(name="drampool", bufs=1, space="DRAM") as drampool:
            A = persist.tile([128, NCHUNK, BL, NUM_UNIT, UNIT_SIZE], bf16)
            Aap = A[:]
            pstA = Aap.ap[0][0]
            b_ij = persist.tile([128, NCHUNK, NUM_UNIT], f32)
            uv = persist.tile([128, NCHUNK, NUM_UNIT], f32)
            vb = persist.tile([128, BL, NUM_UNIT, UNIT_SIZE], bf16)
            pstVB = vb[:].ap[0][0]
            sd = persist.tile([128, BL], bf16)
            s0 = persist.tile([BL, NS], f32)     # iter-0 s_j, SBUF copy
            nc.gpsimd.memset(b_ij[:], 0.0)
            nc.sync.dma_start(sd[:], sd_t[:])

            # ---------------- production + iter-0 s_j ----------------
            CHW = NGRP * 128 * 256          # uhd elements per chunk
            uhd = drampool.tile([NCHUNK, NGRP, 8, BL, 256], bf16,
                                name="uhd")  # rows (gg, cc, b)
            with tc.tile_pool(name="bdp", bufs=2) as bdp, \
                 tc.tile_pool(name="wp", bufs=2) as wp, \
                 tc.tile_pool(name="stgp", bufs=2) as stgp, \
                 tc.tile_pool(name="psp", bufs=3, space="PSUM") as psp, \
                 tc.tile_pool(name="ps0", bufs=1, space="PSUM") as ps0:
                psj0 = ps0.tile([BL, NS], f32)   # s_j iter0: [b, (n,s)]
                for sg in range(NCHUNK):
                    # dense host-built block-diag xT: bd[(cc,i), (gg, cc*16+b)]
                    bd = bdp.tile([128, NGRP, 128], bf16, tag="bd",
                                  name=f"bd_{sg}")
                    pstB = bd[:].ap[0][0]
                    nc.sync.dma_start(
                        sub(bd[:], 0, [[pstB, 128], [1, NGRP * 128]]),
                        sub(xbd_t[:], sg * 128 * NGRP * 128,
                            [[NGRP * 128, 128], [1, NGRP * 128]]))
                    # weights for the chunk: wt[(cc,i), gq, (n,s)]
                    wt = wp.tile([128, NGRP, 256], bf16, tag="wt",
                                 name=f"wt_{sg}")
                    nc.sync.dma_start(
                        wt[:], sub(wr_t[:], sg * NGRP * 128 * 256,
                                   [[256, 128], [128 * 256, NGRP], [1, 256]]))
                    stg = stgp.tile([128, NGRP, 256], bf16, tag="stg",
                                    name=f"stg_{sg}")
                    pstS = stg[:].ap[0][0]
                    for q in range(4):      # 4 psum tiles of 4 groups
                        ps = psp.tile([128, 4 * 256], f32, tag="pp",
                                      name=f"pp_{sg}_{q}")
                        for g4 in range(4):
                            gg = q * 4 + g4
                            nc.tensor.matmul(ps[:, g4 * 256:(g4 + 1) * 256],
                                             bd[:, gg, :], wt[:, gg, :],
                                             start=True, stop=True)
                        eng = nc.vector if q % 2 == 0 else nc.scalar
                        if q % 2 == 0:
                            eng.tensor_copy(
                                sub(stg[:], q * 4 * 256,
                                    [[pstS, 128], [1, 1024]]),
                                ps[:])
                        else:
                            eng.copy(
                                sub(stg[:], q * 4 * 256,
                                    [[pstS, 128], [1, 1024]]),
                                ps[:])
                    # iter-0 s_j partials: psj0[b,(n,s)] += sd^T @ stg
                    for gg in range(NGRP):
                        nc.tensor.matmul(psj0[:], sd[:], stg[:, gg, :],
                                         start=(sg == 0 and gg == 0),
                                         stop=(sg == NCHUNK - 1
                                               and gg == NGRP - 1))
                    # bounce chunk through DRAM to transpose. uhd rows are
                    # (gg, cc, b) so both sides are linear:
                    #   write dst addr = gg*32768 + p*256      (p = cc*16+b)
                    #   read  src addr = p'*4096 + b*256       (p' = gg*8+cc)
                    nc.sync.dma_start(
                        sub(uhd[:], sg * CHW,
                            [[256, 128], [128 * 256, NGRP], [1, 256]]),
                        sub(stg[:], 0, [[pstS, 128], [256, NGRP], [1, 256]]))
                    nc.sync.dma_start(
                        sub(Aap, sg * FREE, [[pstA, 128], [1, FREE]]),
                        sub(uhd[:], sg * CHW, [[FREE, 128], [1, FREE]]))
                # copy iter-0 s_j out of PSUM before the pools close
                nc.vector.tensor_copy(s0[:], psj0[:])

            # ---------------- routing ----------------
            with tc.tile_pool(name="rt", bufs=1) as rt, \
                 tc.tile_pool(name="tb", bufs=2) as tb, \
                 tc.tile_pool(name="pss", bufs=1, space="PSUM") as pss:
                cij = rt.tile([128, NCHUNK, NUM_UNIT], f32)
                cijb = rt.tile([128, NCHUNK, NUM_UNIT], bf16)
                smax = rt.tile([128, NCHUNK], f32)
                ssum = rt.tile([128, NCHUNK], f32)
                ar_sb = rt.tile([128, NCHUNK, NUM_UNIT], f32)
                prodD = rt.tile([128, FREE], bf16)   # DVE agreement scratch
                prodP = rt.tile([128, FREE], bf16)   # Pool agreement scratch
                jch = rt.tile([128, 16], bf16)       # keep-warm gate mirror
                pstPD = prodD[:].ap[0][0]
                pstPP = prodP[:].ap[0][0]
                nc.gpsimd.memset(jch[:].bitcast(f32), 0.0)

                jp = pss.tile([16, 16], f32)         # junk-matmul PSUM target

                def junk(src_ap):
                    """tiny keep-warm matmul; holds the PE clock streak."""
                    if JUNK_ON:
                        nc.tensor.matmul(jp[:], sd[:], src_ap,
                                         start=True, stop=True)

                def jgate(src_f32_ap, tag):
                    """mirror a just-computed f32 value into bf16 + junk."""
                    p = src_f32_ap.ap[0][1]
                    nc.vector.tensor_copy(
                        sub(jch[:], 0, [[jch[:].ap[0][0], p], [1, 16]]),
                        src_f32_ap)
                    junk(jch[:])

                def agr_chunk_dve(k):
                    A_k = sub(Aap, k * FREE, [[pstA, 128], [1, FREE]])
                    nc.vector.tensor_tensor(prodD[:], A_k, vb[:], op=ALU.mult)
                    junk(prodD[:, 2048:2064])
                    for sz in (2048, 1024, 512, 256):
                        nc.vector.tensor_tensor(
                            sub(prodD[:], 0, [[pstPD, 128], [1, sz]]),
                            sub(prodD[:], 0, [[pstPD, 128], [1, sz]]),
                            sub(prodD[:], sz, [[pstPD, 128], [1, sz]]),
                            op=ALU.add)
                        if sz == 2048:
                            junk(prodD[:, 1024:1040])
                        if sz == 512:
                            junk(prodD[:, 256:272])
                    nc.vector.tensor_reduce(
                        uv[:, k, :],
                        sub(prodD[:], 0, [[pstPD, 128], [16, 16], [1, 16]]),
                        axis=AX.X, op=ALU.add)
                    junk(prodD[:, 2064:2080])

                def agr_chunk_pool(k):
                    A_k = sub(Aap, k * FREE, [[pstA, 128], [1, FREE]])
                    nc.gpsimd.tensor_tensor(prodP[:], A_k, vb[:], op=ALU.mult)
                    for sz in (2048, 1024, 512, 256):
                        nc.gpsimd.tensor_tensor(
                            sub(prodP[:], 0, [[pstPP, 128], [1, sz]]),
                            sub(prodP[:], 0, [[pstPP, 128], [1, sz]]),
                            sub(prodP[:], sz, [[pstPP, 128], [1, sz]]),
                            op=ALU.add)
                    for sz in (8, 4, 2):
                        nc.gpsimd.tensor_tensor(
                            sub(prodP[:], 0, [[pstPP, 128], [16, 16], [1, sz]]),
                            sub(prodP[:], 0, [[pstPP, 128], [16, 16], [1, sz]]),
                            sub(prodP[:], sz, [[pstPP, 128], [16, 16],
                                               [1, sz]]),
                            op=ALU.add)
                    nc.gpsimd.tensor_tensor(
                        uv[:, k, :],
                        sub(prodP[:], 0, [[pstPP, 128], [16, 16], [1, 1]]),
                        sub(prodP[:], 1, [[pstPP, 128], [16, 16], [1, 1]]),
                        op=ALU.add)

                def ar_half(rnd, hi, half):
                    """AllReduce of uv chunks `half` (DMA + collective only)."""
                    k0, nk = half[0], len(half)
                    w = nk * NUM_UNIT
                    arbi = drampool.tile([128, w], f32, name=f"arbi_{rnd}_{hi}",
                                         tag=f"arbi{rnd}{hi}")
                    arbo = drampool.tile([128, w], f32, addr_space="Shared",
                                         name=f"arbo_{rnd}_{hi}",
                                         tag=f"arbo{rnd}{hi}")
                    nc.gpsimd.dma_start(
                        arbi[:], sub(uv[:], k0 * NUM_UNIT,
                                     [[uv[:].ap[0][0], 128], [1, w]]))
                    if single_core:
                        nc.gpsimd.dma_start(arbo[:], arbi[:])
                    else:
                        nc.gpsimd.collective_compute(
                            "AllReduce", ALU.add,
                            replica_groups=[list(range(NCORES))],
                            ins=[arbi.opt()], outs=[arbo.opt()])
                    nc.sync.dma_start(
                        sub(ar_sb[:], k0 * NUM_UNIT,
                            [[ar_sb[:].ap[0][0], 128], [1, w]]),
                        arbo[:])

                def softmax_half(rnd, hi, half):
                    k0, nk = half[0], len(half)
                    w = nk * NUM_UNIT
                    jgate(sub(ar_sb[:], k0 * NUM_UNIT,
                              [[ar_sb[:].ap[0][0], 128], [1, 16]]),
                          f"jar{rnd}{hi}")
                    bsl = sub(b_ij[:], k0 * NUM_UNIT,
                              [[b_ij[:].ap[0][0], 128], [1, w]])
                    nc.vector.scalar_tensor_tensor(
                        bsl, sub(ar_sb[:], k0 * NUM_UNIT,
                                 [[ar_sb[:].ap[0][0], 128], [1, w]]),
                        1.0 / B, bsl, op0=ALU.mult, op1=ALU.add)
                    csl = lambda t: sub(t[:], k0 * NUM_UNIT,
                                        [[t[:].ap[0][0], 128], [16, nk],
                                         [1, 16]])
                    ssl = lambda t: sub(t[:], k0,
                                        [[t[:].ap[0][0], 128], [1, nk]])
                    nc.vector.tensor_reduce(ssl(smax), csl(b_ij), axis=AX.X,
                                            op=ALU.max)
                    nc.vector.tensor_tensor(
                        csl(cij), csl(b_ij),
                        sub(smax[:], k0, [[smax[:].ap[0][0], 128], [1, nk],
                                          [0, 16]]),
                        op=ALU.subtract)
                    nc.scalar.activation(csl(cij), csl(cij), ACT.Exp)
                    nc.vector.tensor_reduce(ssl(ssum), csl(cij), axis=AX.X,
                                            op=ALU.add)
                    nc.vector.reciprocal(ssl(ssum), ssl(ssum))
                    nc.vector.tensor_tensor(
                        csl(cij), csl(cij),
                        sub(ssum[:], k0, [[ssum[:].ap[0][0], 128], [1, nk],
                                          [0, 16]]),
                        op=ALU.mult)
                    nc.vector.tensor_copy(csl(cijb), csl(cij))
                    junk(cijb[:, k0, :])

                def sj_mms(psjH_t, hb, ks):
                    for k in ks:
                        for j in range(4):
                            nc.tensor.matmul(
                                psjH_t[:, j * 512:(j + 1) * 512],
                                cijb[:, k, :],
                                sub(Aap, k * FREE + hb * 2048 + j * 512,
                                    [[pstA, 128], [1, 512]]),
                                start=(k == 0), stop=(k == NCHUNK - 1))

                def squash(src_ap, pst_src, nb, final, rnd, hb, order_nbs):
                    """squash over s of src [16, (x, s)], x of size nb.
                    order_nbs: src rows are n (True) or b (False)."""
                    W = nb * 16
                    s2 = tb.tile([16, W], f32, tag="s2", name=f"s2_{rnd}_{hb}")
                    nc.vector.tensor_tensor(s2[:], src_ap, src_ap,
                                            op=ALU.mult)
                    sq = tb.tile([16, nb], f32, tag="sq",
                                 name=f"sq_{rnd}_{hb}")
                    nc.vector.tensor_reduce(
                        sq[:], sub(s2[:], 0, [[s2[:].ap[0][0], 16], [16, nb],
                                              [1, 16]]),
                        axis=AX.X, op=ALU.add)
                    jgate(sub(sq[:], 0, [[sq[:].ap[0][0], 16], [1, nb],
                                         [0, 16 // nb]]),
                          f"jsq{rnd}{hb}")
                    rsq = tb.tile([16, nb], f32, tag="rsq",
                                  name=f"rsq_{rnd}_{hb}")
                    nc.scalar.sqrt(rsq[:], sq[:])
                    den = tb.tile([16, nb], f32, tag="den",
                                  name=f"den_{rnd}_{hb}")
                    nc.vector.scalar_tensor_tensor(den[:], sq[:], 1.0, rsq[:],
                                                   op0=ALU.add, op1=ALU.mult)
                    nc.vector.reciprocal(den[:], den[:])
                    fac = tb.tile([16, nb], f32, tag="fac",
                                  name=f"fac_{rnd}_{hb}")
                    nc.vector.tensor_tensor(fac[:], sq[:], den[:], op=ALU.mult)
                    pstF = fac[:].ap[0][0]
                    fb = sub(fac[:], 0, [[pstF, 16], [1, nb], [0, 16]])
                    if final:
                        v32 = tb.tile([16, W], f32, tag="v32",
                                      name=f"v32_{hb}")
                        nc.vector.tensor_tensor(v32[:], src_ap, fb,
                                                op=ALU.mult)
                        nc.sync.dma_start(
                            sub(vout_t[:], hb * 128,
                                [[256, 16], [1, W]]),
                            v32[:])
                        return
                    v16 = tb.tile([16, W], bf16, tag="v16",
                                  name=f"v16_{rnd}_{hb}")
                    nc.vector.tensor_tensor(v16[:], src_ap, fb, op=ALU.mult)
                    jgate(sub(fac[:], 0, [[pstF, 16], [1, nb], [0, 16 // nb]]),
                          f"jv{rnd}{hb}")
                    # flatten to DRAM in (b, n, s) order, then broadcast
                    vfl = drampool.tile([FREE], bf16, name=f"vfl_{rnd}_{hb}",
                                        tag=f"vfl{rnd}{hb}")
                    pstV = v16[:].ap[0][0]
                    if order_nbs:    # v16 is [n, (b_half, s)]
                        nc.sync.dma_start(
                            sub(vfl[:], hb * 8 * 256,
                                [[16, 16], [256, nb], [1, 16]]),
                            sub(v16[:], 0, [[pstV, 16], [16, nb], [1, 16]]))
                    else:            # v16 is [b, (n, s)] (iter 0, full)
                        nc.sync.dma_start(
                            sub(vfl[:], 0, [[256, 16], [1, 256]]),
                            sub(v16[:], 0, [[pstV, 16], [1, 256]]))
                    nc.sync.dma_start(
                        sub(vb[:], hb * 8 * 256 if order_nbs else 0,
                            [[pstVB, 128], [1, W * 16 if order_nbs else FREE]]),
                        sub(vfl[:], hb * 8 * 256 if order_nbs else 0,
                            [[0, 128], [1, W * 16 if order_nbs else FREE]]))

                def diag_squash_half(rnd, hb, psjH_t, final):
                    sjf = tb.tile([NUM_UNIT, FREE // 2], f32, tag="sjf",
                                  bufs=1, name=f"sjf_{rnd}_{hb}")
                    pstJ = sjf[:].ap[0][0]
                    nc.vector.tensor_copy(sjf[:, :1024], psjH_t[:, :1024])
                    nc.scalar.copy(sjf[:, 1024:], psjH_t[:, 1024:])
                    jgate(sub(sjf[:], 0, [[pstJ, 16], [1, 16]]),
                          f"jsjf{rnd}{hb}")
                    s_t = tb.tile([NUM_UNIT, 8, UNIT_SIZE], f32, tag="s_t",
                                  name=f"s_t{rnd}_{hb}")
                    nc.sync.dma_start(
                        s_t[:], sub(sjf[:], 0, [[pstJ + 16, 16], [256, 8],
                                                [1, 16]]))
                    squash(sub(s_t[:], 0, [[s_t[:].ap[0][0], 16], [1, 128]]),
                           s_t[:].ap[0][0], nb=8, final=final, rnd=rnd, hb=hb,
                           order_nbs=True)

                # iter-0 squash from s0 [b, (n,s)]
                squash(sub(s0[:], 0, [[s0[:].ap[0][0], 16], [1, 256]]),
                       s0[:].ap[0][0], nb=16, final=(niters == 1), rnd=0,
                       hb=0, order_nbs=False)

                HALF1 = (0, 1, 2, 3, 4)
                HALF2 = (5, 6, 7, 8)
                POOLK = (7, 8)
                DVE1 = tuple(k for k in HALF1 if k not in POOLK)
                DVE2 = tuple(k for k in HALF2 if k not in POOLK)

                for rnd in range(1, niters):
                    final = rnd == niters - 1
                    agr_chunk_pool(POOLK[0])
                    agr_chunk_pool(POOLK[1])
                    for k in DVE1:
                        agr_chunk_dve(k)
                    ar_half(rnd, 0, HALF1)
                    for k in DVE2:
                        agr_chunk_dve(k)
                    ar_half(rnd, 1, HALF2)
                    psjH1 = pss.tile([NUM_UNIT, FREE // 2], f32, tag="psjH",
                                     name=f"psjH_{rnd}_0")
                    softmax_half(rnd, 0, HALF1)
                    sj_mms(psjH1, 0, HALF1)
                    softmax_half(rnd, 1, HALF2)
                    sj_mms(psjH1, 0, HALF2)
                    diag_squash_half(rnd, 0, psjH1, final)
                    psjH2 = pss.tile([NUM_UNIT, FREE // 2], f32, tag="psjH",
                                     name=f"psjH_{rnd}_1")
                    sj_mms(psjH2, 1, HALF1 + HALF2)
                    diag_squash_half(rnd, 1, psjH2, final)

    nc.compile()
    return nc


def _prep(x, weight):
    import ml_dtypes
    bf = ml_dtypes.bfloat16
    wr = np.ascontiguousarray(
        weight.reshape(NCHUNK * NGRP, 8, NUM_UNIT, UNIT_SIZE, IN_UNIT)
        .transpose(0, 1, 4, 2, 3).reshape(NCHUNK * NGRP * 128, 256)
    ).astype(bf)
    sd = (np.tile(np.eye(BL, dtype=np.float32), (8, 1)) / NUM_UNIT).astype(bf)
    in_maps = []
    for c in range(NCORES):
        xs = x[c * BL:(c + 1) * BL]          # [BL, i, C]
        # xv[sg, cc, i, gg, b] = x[b, i, sg*128 + gg*8 + cc]
        xv = xs.reshape(BL, IN_UNIT, NCHUNK, NGRP, 8).transpose(2, 4, 1, 3, 0)
        # dense block-diag: xbd[sg, (cc,i), gg, (cc2,b)], nonzero iff cc2==cc
        xbd = np.zeros((NCHUNK, 8, IN_UNIT, NGRP, 8, BL), np.float32)
        for cc in range(8):
            xbd[:, cc, :, :, cc, :] = xv[:, cc]
        xbd = np.ascontiguousarray(
            xbd.reshape(NCHUNK, 128, NGRP * 128)).astype(bf)
        in_maps.append({"wr": wr, "xbd": xbd, "sd": sd})
    return in_maps


def kernel(x, x_original, weight, mode, epoch, _trace=False):
    from concourse.bass_utils import run_bass_kernel_spmd

    x = np.asarray(x, dtype=np.float32)
    weight = np.asarray(weight, dtype=np.float32)
    if "nc" not in _cache:
        _cache["nc"] = _build()
    nc = _cache["nc"]
    in_maps = _prep(x, weight)
    res = run_bass_kernel_spmd(nc, in_maps, core_ids=list(range(NCORES)),
                               trace=_trace)
    _cache["last_result"] = res
    out = np.empty((B, NUM_UNIT, UNIT_SIZE), np.float32)
    for c in range(NCORES):
        vo = res.results[c]["vout"].reshape(NUM_UNIT, BL, UNIT_SIZE)
        out[c * BL:(c + 1) * BL] = vo.transpose(1, 0, 2)
    return out[..., None]


# revision 23
# speedup vs baseline: 1.0262x; 1.0262x over previous
"""CapsuleLayer dynamic-routing kernel for 8 Trainium2 NeuronCores.

Sharding: data-parallel over batch (16 batches/core), weight replicated.
  u_hat[b,c,n,s] = sum_i W[c,n,s,i] * x[b,i,c]   (PE, bf16, block-diag x)
  3 routing iterations; b_ij update takes a mean over the full batch via
  AllReduce (split in channel-halves so it overlaps compute).

v2 design vs baseline:
  - everything bf16 (tol 2e-2; measured rel err ~1e-3): halves DMA + 2x DVE
  - u_hat transposed to [c-part, (b,n,s)] via one SBUF->SBUF DMA per chunk
    (512B runs) instead of a DRAM round-trip
  - iteration-0 s_j computed during production from the staging tiles with a
    delta/16 stationary (PE idle time), output directly [b,(n,s)] (no diag)
  - agreement = DVE tensor_tensor mult + pairwise fold tree (bf16 2x mode);
    3 chunk-mults offloaded to GPSIMD
  - diag extract for s_j via a single strided DVE copy from PSUM
  - v broadcast via DRAM bounce + stride-0 broadcast DMA
  - tiny keep-warm matmuls gated on DVE steps hold the PE clock at max
"""

import sys

sys.path.insert(0, "/opt/trn_rl_repo")

import numpy as np

B, IN_UNIT, IN_CHANNEL = 128, 16, 1152
NUM_UNIT, UNIT_SIZE = 16, 16
NCORES = 8
BL = B // NCORES               # 16 batches per core
NCHUNK = IN_CHANNEL // 128     # 9 c-chunks
NGRP = 16                      # groups of 8 channels per chunk
NS = NUM_UNIT * UNIT_SIZE      # 256
FREE = BL * NS                 # 4096 = (b, n, s) free size per chunk
POOL_CHUNKS = (6, 7, 8)        # agreement mults done on gpsimd
JUNK_ON = True                 # keep-warm matmuls
SPLIT_AR = True                # two collectives per round

_cache = {}


def _build(single_core=False, niters=3):
    import concourse.bass as bass
    import concourse.bacc as bacc
    import concourse.mybir as mybir
    import concourse.tile as tile

    f32 = mybir.dt.float32
    bf16 = mybir.dt.bfloat16
    ALU = mybir.AluOpType
    AX = mybir.AxisListType
    ACT = mybir.ActivationFunctionType

    def sub(ap, off, dims, cast=None):
        a = bass.AP(ap.tensor, ap.offset + off, [list(d) for d in dims])
        return a.bitcast(cast) if cast is not None else a

    nc = bacc.Bacc("TRN2", target_bir_lowering=False, debug=False,
                   num_devices=1 if single_core else NCORES)

    wr_t = nc.dram_tensor("wr", [NCHUNK * NGRP * 128, 256], bf16,
                          kind="ExternalInput")
    xc2_t = nc.dram_tensor("xc2", [NCHUNK, IN_UNIT, NGRP, 8, BL], bf16,
                           kind="ExternalInput")
    msk_t = nc.dram_tensor("msk", [128, NGRP * 128], bf16,
                           kind="ExternalInput")
    sd_t = nc.dram_tensor("sd", [128, BL], bf16, kind="ExternalInput")
    vout_t = nc.dram_tensor("vout", [NUM_UNIT, BL * UNIT_SIZE], f32,
                            kind="ExternalOutput")

    with tile.TileContext(nc) as tc:
        with tc.tile_pool(name="persist", bufs=1) as persist, \
             tc.tile_pool(name="drampool", bufs=1, space="DRAM") as drampool:
            A = persist.tile([128, NCHUNK, BL, NUM_UNIT, UNIT_SIZE], bf16)
            Aap = A[:]
            pstA = Aap.ap[0][0]
            b_ij = persist.tile([128, NCHUNK, NUM_UNIT], f32)
            uv = persist.tile([128, NCHUNK, NUM_UNIT], f32)
            vb = persist.tile([128, BL, NUM_UNIT, UNIT_SIZE], bf16)
            pstVB = vb[:].ap[0][0]
            sd = persist.tile([128, BL], bf16)
            s0 = persist.tile([BL, NS], f32)     # iter-0 s_j, SBUF copy
            msk = persist.tile([128, NGRP * 128], bf16)  # block-diag mask
            nc.gpsimd.memset(b_ij[:], 0.0)
            nc.sync.dma_start(sd[:], sd_t[:])
            nc.sync.dma_start(msk[:], msk_t[:])

            # ---------------- production + iter-0 s_j ----------------
            CHW = NGRP * 128 * 256          # uhd elements per chunk
            uhd = drampool.tile([NCHUNK, NGRP, 8, BL, 256], bf16,
                                name="uhd")  # rows (gg, cc, b)
            with tc.tile_pool(name="bdp", bufs=2) as bdp, \
                 tc.tile_pool(name="wp", bufs=2) as wp, \
                 tc.tile_pool(name="stgp", bufs=2) as stgp, \
                 tc.tile_pool(name="psp", bufs=3, space="PSUM") as psp, \
                 tc.tile_pool(name="ps0", bufs=1, space="PSUM") as ps0:
                psj0 = ps0.tile([BL, NS], f32)   # s_j iter0: [b, (n,s)]
                for sg in range(NCHUNK):
                    # x replicated over cc via stride-0 DRAM read, then
                    # masked to block-diag: bd[(cc,i), (gg, cc*16+b)]
                    bd = bdp.tile([128, NGRP, 128], bf16, tag="bd",
                                  name=f"bd_{sg}")
                    pstB = bd[:].ap[0][0]
                    nc.sync.dma_start(
                        sub(bd[:], 0, [[pstB, 128], [1, NGRP * 128]]),
                        sub(xc2_t[:], sg * IN_UNIT * NGRP * 128,
                            [[0, 8], [NGRP * 128, IN_UNIT],
                             [1, NGRP * 128]]))
                    nc.vector.tensor_tensor(
                        sub(bd[:], 0, [[pstB, 128], [1, NGRP * 128]]),
                        sub(bd[:], 0, [[pstB, 128], [1, NGRP * 128]]),
                        msk[:], op=ALU.mult)
                    # weights for the chunk: wt[(cc,i), gq, (n,s)]
                    wt = wp.tile([128, NGRP, 256], bf16, tag="wt",
                                 name=f"wt_{sg}")
                    nc.sync.dma_start(
                        wt[:], sub(wr_t[:], sg * NGRP * 128 * 256,
                                   [[256, 128], [128 * 256, NGRP], [1, 256]]))
                    stg = stgp.tile([128, NGRP, 256], bf16, tag="stg",
                                    name=f"stg_{sg}")
                    pstS = stg[:].ap[0][0]
                    for q in range(4):      # 4 psum tiles of 4 groups
                        ps = psp.tile([128, 4 * 256], f32, tag="pp",
                                      name=f"pp_{sg}_{q}")
                        for g4 in range(4):
                            gg = q * 4 + g4
                            nc.tensor.matmul(ps[:, g4 * 256:(g4 + 1) * 256],
                                             bd[:, gg, :], wt[:, gg, :],
                                             start=True, stop=True)
                        eng = nc.vector if q % 2 == 0 else nc.scalar
                        if q % 2 == 0:
                            eng.tensor_copy(
                                sub(stg[:], q * 4 * 256,
                                    [[pstS, 128], [1, 1024]]),
                                ps[:])
                        else:
                            eng.copy(
                                sub(stg[:], q * 4 * 256,
                                    [[pstS, 128], [1, 1024]]),
                                ps[:])
                    # iter-0 s_j partials: psj0[b,(n,s)] += sd^T @ stg
                    for gg in range(NGRP):
                        nc.tensor.matmul(psj0[:], sd[:], stg[:, gg, :],
                                         start=(sg == 0 and gg == 0),
                                         stop=(sg == NCHUNK - 1
                                               and gg == NGRP - 1))
                    # bounce chunk through DRAM to transpose. uhd rows are
                    # (gg, cc, b) so both sides are linear:
                    #   write dst addr = gg*32768 + p*256      (p = cc*16+b)
                    #   read  src addr = p'*4096 + b*256       (p' = gg*8+cc)
                    nc.sync.dma_start(
                        sub(uhd[:], sg * CHW,
                            [[256, 128], [128 * 256, NGRP], [1, 256]]),
                        sub(stg[:], 0, [[pstS, 128], [256, NGRP], [1, 256]]))
                    nc.sync.dma_start(
                        sub(Aap, sg * FREE, [[pstA, 128], [1, FREE]]),
                        sub(uhd[:], sg * CHW, [[FREE, 128], [1, FREE]]))
                # copy iter-0 s_j out of PSUM before the pools close
                nc.vector.tensor_copy(s0[:], psj0[:])

            # ---------------- routing ----------------
            with tc.tile_pool(name="rt", bufs=1) as rt, \
                 tc.tile_pool(name="tb", bufs=2) as tb, \
                 tc.tile_pool(name="pss", bufs=1, space="PSUM") as pss:
                cij = rt.tile([128, NCHUNK, NUM_UNIT], f32)
                cijb = rt.tile([128, NCHUNK, NUM_UNIT], bf16)
                smax = rt.tile([128, NCHUNK], f32)
                ssum = rt.tile([128, NCHUNK], f32)
                ar_sb = rt.tile([128, NCHUNK, NUM_UNIT], f32)
                prodD = rt.tile([128, FREE], bf16)   # DVE agreement scratch
                prodP = rt.tile([128, FREE], bf16)   # Pool agreement scratch
                jch = rt.tile([128, 16], bf16)       # keep-warm gate mirror
                pstPD = prodD[:].ap[0][0]
                pstPP = prodP[:].ap[0][0]
                nc.gpsimd.memset(jch[:].bitcast(f32), 0.0)

                jp = pss.tile([16, 16], f32)         # junk-matmul PSUM target

                def junk(src_ap):
                    """tiny keep-warm matmul; holds the PE clock streak."""
                    if JUNK_ON:
                        nc.tensor.matmul(jp[:], sd[:], src_ap,
                                         start=True, stop=True)

                def jgate(src_f32_ap, tag):
                    """mirror a just-computed f32 value into bf16 + junk."""
                    p = src_f32_ap.ap[0][1]
                    nc.vector.tensor_copy(
                        sub(jch[:], 0, [[jch[:].ap[0][0], p], [1, 16]]),
                        src_f32_ap)
                    junk(jch[:])

                def agr_chunk_dve(k):
                    A_k = sub(Aap, k * FREE, [[pstA, 128], [1, FREE]])
                    nc.vector.tensor_tensor(prodD[:], A_k, vb[:], op=ALU.mult)
                    junk(prodD[:, 2048:2064])
                    for sz in (2048, 1024, 512, 256):
                        nc.vector.tensor_tensor(
                            sub(prodD[:], 0, [[pstPD, 128], [1, sz]]),
                            sub(prodD[:], 0, [[pstPD, 128], [1, sz]]),
                            sub(prodD[:], sz, [[pstPD, 128], [1, sz]]),
                            op=ALU.add)
                        if sz == 2048:
                            junk(prodD[:, 1024:1040])
                        if sz == 512:
                            junk(prodD[:, 256:272])
                    nc.vector.tensor_reduce(
                        uv[:, k, :],
                        sub(prodD[:], 0, [[pstPD, 128], [16, 16], [1, 16]]),
                        axis=AX.X, op=ALU.add)
                    junk(prodD[:, 2064:2080])

                def agr_chunk_pool(k):
                    A_k = sub(Aap, k * FREE, [[pstA, 128], [1, FREE]])
                    nc.gpsimd.tensor_tensor(prodP[:], A_k, vb[:], op=ALU.mult)
                    for sz in (2048, 1024, 512, 256):
                        nc.gpsimd.tensor_tensor(
                            sub(prodP[:], 0, [[pstPP, 128], [1, sz]]),
                            sub(prodP[:], 0, [[pstPP, 128], [1, sz]]),
                            sub(prodP[:], sz, [[pstPP, 128], [1, sz]]),
                            op=ALU.add)
                    for sz in (8, 4, 2):
                        nc.gpsimd.tensor_tensor(
                            sub(prodP[:], 0, [[pstPP, 128], [16, 16], [1, sz]]),
                            sub(prodP[:], 0, [[pstPP, 128], [16, 16], [1, sz]]),
                            sub(prodP[:], sz, [[pstPP, 128], [16, 16],
                                               [1, sz]]),
                            op=ALU.add)
                    nc.gpsimd.tensor_tensor(
                        uv[:, k, :],
                        sub(prodP[:], 0, [[pstPP, 128], [16, 16], [1, 1]]),
                        sub(prodP[:], 1, [[pstPP, 128], [16, 16], [1, 1]]),
                        op=ALU.add)

                def ar_half(rnd, hi, half):
                    """AllReduce of uv chunks `half` (DMA + collective only)."""
                    k0, nk = half[0], len(half)
                    w = nk * NUM_UNIT
                    arbi = drampool.tile([128, w], f32, name=f"arbi_{rnd}_{hi}",
                                         tag=f"arbi{rnd}{hi}")
                    arbo = drampool.tile([128, w], f32, addr_space="Shared",
                                         name=f"arbo_{rnd}_{hi}",
                                         tag=f"arbo{rnd}{hi}")
                    nc.gpsimd.dma_start(
                        arbi[:], sub(uv[:], k0 * NUM_UNIT,
                                     [[uv[:].ap[0][0], 128], [1, w]]))
                    if single_core:
                        nc.gpsimd.dma_start(arbo[:], arbi[:])
                    else:
                        nc.gpsimd.collective_compute(
                            "AllReduce", ALU.add,
                            replica_groups=[list(range(NCORES))],
                            ins=[arbi.opt()], outs=[arbo.opt()])
                    nc.sync.dma_start(
                        sub(ar_sb[:], k0 * NUM_UNIT,
                            [[ar_sb[:].ap[0][0], 128], [1, w]]),
                        arbo[:])

                def softmax_half(rnd, hi, half):
                    k0, nk = half[0], len(half)
                    w = nk * NUM_UNIT
                    jgate(sub(ar_sb[:], k0 * NUM_UNIT,
                              [[ar_sb[:].ap[0][0], 128], [1, 16]]),
                          f"jar{rnd}{hi}")
                    bsl = sub(b_ij[:], k0 * NUM_UNIT,
                              [[b_ij[:].ap[0][0], 128], [1, w]])
                    nc.vector.scalar_tensor_tensor(
                        bsl, sub(ar_sb[:], k0 * NUM_UNIT,
                                 [[ar_sb[:].ap[0][0], 128], [1, w]]),
                        1.0 / B, bsl, op0=ALU.mult, op1=ALU.add)
                    csl = lambda t: sub(t[:], k0 * NUM_UNIT,
                                        [[t[:].ap[0][0], 128], [16, nk],
                                         [1, 16]])
                    ssl = lambda t: sub(t[:], k0,
                                        [[t[:].ap[0][0], 128], [1, nk]])
                    nc.vector.tensor_reduce(ssl(smax), csl(b_ij), axis=AX.X,
                                            op=ALU.max)
                    nc.vector.tensor_tensor(
                        csl(cij), csl(b_ij),
                        sub(smax[:], k0, [[smax[:].ap[0][0], 128], [1, nk],
                                          [0, 16]]),
                        op=ALU.subtract)
                    nc.scalar.activation(csl(cij), csl(cij), ACT.Exp)
                    nc.vector.tensor_reduce(ssl(ssum), csl(cij), axis=AX.X,
                                            op=ALU.add)
                    nc.vector.reciprocal(ssl(ssum), ssl(ssum))
                    nc.vector.tensor_tensor(
                        csl(cij), csl(cij),
                        sub(ssum[:], k0, [[ssum[:].ap[0][0], 128], [1, nk],
                                          [0, 16]]),
                        op=ALU.mult)
                    nc.vector.tensor_copy(csl(cijb), csl(cij))
                    junk(cijb[:, k0, :])

                def sj_mms(psj_t, qb, ks):
                    for k in ks:
                        for j in range(2):
                            nc.tensor.matmul(
                                psj_t[:, j * 512:(j + 1) * 512],
                                cijb[:, k, :],
                                sub(Aap, k * FREE + qb * 1024 + j * 512,
                                    [[pstA, 128], [1, 512]]),
                                start=(k == 0), stop=(k == NCHUNK - 1))

                def squash(src_ap, pst_src, nb, final, rnd, hb, order_nbs):
                    """squash over s of src [16, (x, s)], x of size nb.
                    order_nbs: src rows are n (True) or b (False)."""
                    W = nb * 16
                    s2 = tb.tile([16, W], f32, tag="s2", name=f"s2_{rnd}_{hb}")
                    nc.vector.tensor_tensor(s2[:], src_ap, src_ap,
                                            op=ALU.mult)
                    sq = tb.tile([16, nb], f32, tag="sq",
                                 name=f"sq_{rnd}_{hb}")
                    nc.vector.tensor_reduce(
                        sq[:], sub(s2[:], 0, [[s2[:].ap[0][0], 16], [16, nb],
                                              [1, 16]]),
                        axis=AX.X, op=ALU.add)
                    jgate(sub(sq[:], 0, [[sq[:].ap[0][0], 16], [1, nb],
                                         [0, 16 // nb]]),
                          f"jsq{rnd}{hb}")
                    rsq = tb.tile([16, nb], f32, tag="rsq",
                                  name=f"rsq_{rnd}_{hb}")
                    nc.scalar.sqrt(rsq[:], sq[:])
                    den = tb.tile([16, nb], f32, tag="den",
                                  name=f"den_{rnd}_{hb}")
                    nc.vector.scalar_tensor_tensor(den[:], sq[:], 1.0, rsq[:],
                                                   op0=ALU.add, op1=ALU.mult)
                    nc.vector.reciprocal(den[:], den[:])
                    fac = tb.tile([16, nb], f32, tag="fac",
                                  name=f"fac_{rnd}_{hb}")
                    nc.vector.tensor_tensor(fac[:], sq[:], den[:], op=ALU.mult)
                    pstF = fac[:].ap[0][0]
                    fb = sub(fac[:], 0, [[pstF, 16], [1, nb], [0, 16]])
                    if final:
                        v32 = tb.tile([16, W], f32, tag="v32",
                                      name=f"v32_{hb}")
                        nc.vector.tensor_tensor(v32[:], src_ap, fb,
                                                op=ALU.mult)
                        nc.sync.dma_start(
                            sub(vout_t[:], hb * W,
                                [[256, 16], [1, W]]),
                            v32[:])
                        return
                    v16 = tb.tile([16, W], bf16, tag="v16",
                                  name=f"v16_{rnd}_{hb}")
                    nc.vector.tensor_tensor(v16[:], src_ap, fb, op=ALU.mult)
                    jgate(sub(fac[:], 0, [[pstF, 16], [1, nb], [0, 16 // nb]]),
                          f"jv{rnd}{hb}")
                    # flatten to DRAM in (b, n, s) order, then broadcast
                    vfl = drampool.tile([FREE], bf16, name=f"vfl_{rnd}_{hb}",
                                        tag=f"vfl{rnd}{hb}")
                    pstV = v16[:].ap[0][0]
                    boff = hb * nb * 256
                    if order_nbs:    # v16 is [n, (b_part, s)]
                        nc.sync.dma_start(
                            sub(vfl[:], boff,
                                [[16, 16], [256, nb], [1, 16]]),
                            sub(v16[:], 0, [[pstV, 16], [16, nb], [1, 16]]))
                    else:            # v16 is [b, (n, s)] (iter 0, full)
                        nc.sync.dma_start(
                            sub(vfl[:], 0, [[256, 16], [1, 256]]),
                            sub(v16[:], 0, [[pstV, 16], [1, 256]]))
                    nc.sync.dma_start(
                        sub(vb[:], boff if order_nbs else 0,
                            [[pstVB, 128],
                             [1, nb * 256 if order_nbs else FREE]]),
                        sub(vfl[:], boff if order_nbs else 0,
                            [[0, 128], [1, nb * 256 if order_nbs else FREE]]))

                def diag_squash_q(rnd, qb, psj_t, final):
                    sjf = tb.tile([NUM_UNIT, 1024], f32, tag="sjf",
                                  bufs=2, name=f"sjf_{rnd}_{qb}")
                    pstJ = sjf[:].ap[0][0]
                    nc.vector.tensor_copy(sjf[:, :512], psj_t[:, :512])
                    nc.scalar.copy(sjf[:, 512:], psj_t[:, 512:])
                    jgate(sub(sjf[:], 0, [[pstJ, 16], [1, 16]]),
                          f"jsjf{rnd}{qb}")
                    s_t = tb.tile([NUM_UNIT, 4, UNIT_SIZE], f32, tag="s_t",
                                  name=f"s_t{rnd}_{qb}")
                    nc.sync.dma_start(
                        s_t[:], sub(sjf[:], 0, [[pstJ + 16, 16], [256, 4],
                                                [1, 16]]))
                    squash(sub(s_t[:], 0, [[s_t[:].ap[0][0], 16], [1, 64]]),
                           s_t[:].ap[0][0], nb=4, final=final, rnd=rnd, hb=qb,
                           order_nbs=True)

                # iter-0 squash from s0 [b, (n,s)]
                squash(sub(s0[:], 0, [[s0[:].ap[0][0], 16], [1, 256]]),
                       s0[:].ap[0][0], nb=16, final=(niters == 1), rnd=0,
                       hb=0, order_nbs=False)

                HALF1 = (0, 1, 2, 3, 4)
                HALF2 = (5, 6, 7, 8)
                POOLK = (7, 8)
                DVE1 = tuple(k for k in HALF1 if k not in POOLK)
                DVE2 = tuple(k for k in HALF2 if k not in POOLK)

                for rnd in range(1, niters):
                    final = rnd == niters - 1
                    agr_chunk_pool(POOLK[0])
                    agr_chunk_pool(POOLK[1])
                    for k in DVE1:
                        agr_chunk_dve(k)
                    ar_half(rnd, 0, HALF1)
                    for k in DVE2:
                        agr_chunk_dve(k)
                    ar_half(rnd, 1, HALF2)
                    softmax_half(rnd, 0, HALF1)
                    psjQ0 = pss.tile([NUM_UNIT, 1024], f32, tag="psjQ",
                                     bufs=2, name=f"psjQ_{rnd}_0")
                    sj_mms(psjQ0, 0, HALF1)
                    softmax_half(rnd, 1, HALF2)
                    sj_mms(psjQ0, 0, HALF2)
                    diag_squash_q(rnd, 0, psjQ0, final)
                    for qb in range(1, 4):
                        psjQ = pss.tile([NUM_UNIT, 1024], f32, tag="psjQ",
                                        bufs=2, name=f"psjQ_{rnd}_{qb}")
                        sj_mms(psjQ, qb, HALF1 + HALF2)
                        diag_squash_q(rnd, qb, psjQ, final)

    nc.compile()
    return nc


def _prep(x, weight):
    import ml_dtypes
    bf = ml_dtypes.bfloat16
    wr = np.ascontiguousarray(
        weight.reshape(NCHUNK * NGRP, 8, NUM_UNIT, UNIT_SIZE, IN_UNIT)
        .transpose(0, 1, 4, 2, 3).reshape(NCHUNK * NGRP * 128, 256)
    ).astype(bf)
    sd = (np.tile(np.eye(BL, dtype=np.float32), (8, 1)) / NUM_UNIT).astype(bf)
    # block-diag mask [ (cc,i), (gg, cc2, b) ] = 1 iff cc2 == cc
    m = np.zeros((8, IN_UNIT, NGRP, 8, BL), np.float32)
    for cc in range(8):
        m[cc, :, :, cc, :] = 1.0
    msk = np.ascontiguousarray(m.reshape(128, NGRP * 128)).astype(bf)
    in_maps = []
    for c in range(NCORES):
        xs = x[c * BL:(c + 1) * BL]          # [BL, i, C]
        # xc2[sg, i, gg, cc, b] = x[b, i, sg*128 + gg*8 + cc]
        xc2 = np.ascontiguousarray(
            xs.reshape(BL, IN_UNIT, NCHUNK, NGRP, 8)
            .transpose(2, 1, 3, 4, 0)).astype(bf)
        in_maps.append({"wr": wr, "xc2": xc2, "sd": sd, "msk": msk})
    return in_maps


def kernel(x, x_original, weight, mode, epoch, _trace=False):
    from concourse.bass_utils import run_bass_kernel_spmd

    x = np.asarray(x, dtype=np.float32)
    weight = np.asarray(weight, dtype=np.float32)
    if "nc" not in _cache:
        _cache["nc"] = _build()
    nc = _cache["nc"]
    in_maps = _prep(x, weight)
    res = run_bass_kernel_spmd(nc, in_maps, core_ids=list(range(NCORES)),
                               trace=_trace)
    _cache["last_result"] = res
    out = np.empty((B, NUM_UNIT, UNIT_SIZE), np.float32)
    for c in range(NCORES):
        vo = res.results[c]["vout"].reshape(NUM_UNIT, BL, UNIT_SIZE)
        out[c * BL:(c + 1) * BL] = vo.transpose(1, 0, 2)
    return out[..., None]
